# Initial kernel scaffold
#
"""
Trainium2 Bass kernel for nn_EquivariantProductBasisBlock.

Math (per node n, channel c):
    s   = feats[n,c,0];  v = feats[n,c,1:4]
    v2  = |v|^2 ;  s2 = s^2
    w0p[n,c] = sum_e attrs[n,e] * w_paths0[e,p,c]   (p = 0..4)
    w1p[n,c] = sum_e attrs[n,e] * w_paths1[e,p,c]   (p = 0..3)
    out0 = s*w00 + s2*(w01 + s*w03) + v2*(w02 + s*w04)      (Horner)
    c1   = w10 + s*w11 + s2*w12 + v2*w13
    y0   = out0 @ W_lin0 ;  y1_i = (c1 * v_i) @ W_lin1
    out  = [y0 | interleave_i(y1_i)]    # col 128 + 3m + i

Implementation notes (tuned from hardware traces; ~302 us HW exec):
  * Nodes padded to 12800/core; chunks of 512 nodes = 4 subtiles x 128.
    128-partition DMA tiles are mandatory: <128-partition transfers fan
    out to only 5/16 SDMA engines (~127 GB/s instead of ~400).
  * Host supplies node_feats in channel-major planes [4, 128, n] and
    node_attrs transposed+duplicated [20, n] (bf16), so the kernel needs
    ZERO on-chip transposes (PE transposes + PSUM staging dominated v1).
  * Path weights via one bf16 matmul per path with K=20: rows 0..9 carry
    bf16_hi(table), rows 10..19 bf16_lo residuals; attrs duplicated.
    Exact to ~1e-5 for one-hot attrs, 1-cycle/row + fast weight load.
  * Final per-irrep linear in float32r (1 pass/matmul instead of fp32's
    2 half-rate passes; ~5e-4 rounding).  Path-sum folded into PSUM
    accumulation; y1 interleaving free via host-expanded W1e [128,3*384].
  * Elementwise split: squares on ScalarE + PSUM->SBUF y copies; leaf-only
    work on GPSIMD (|v|^2 sums, Z_C) so its slow 2-input rate stays off the
    c1 -> Z1 critical chain; the 15 tensor*tensor products/adds on VectorE.
    PSUM: 4 banks rotate path-weight matmul outputs, 4 double-buffer y.
"""

import sys

sys.path.insert(0, "/opt/trn_rl_repo")

from contextlib import ExitStack

import ml_dtypes
import numpy as np

import concourse.bass as bass
import concourse.tile as tile
from concourse import bacc, mybir
from concourse.bass_utils import run_bass_kernel_spmd

N_CORES = 8
N_NODES = 100000
C = 128
S = 10
SUB = 128
NSUB = 4
CHUNK = SUB * NSUB           # 512
PER_CORE = 12800             # 25 chunks; 8*12800 = 102400 (padded)
F32 = mybir.dt.float32
F32R = mybir.dt.float32r
BF16 = mybir.dt.bfloat16


def build_bass(per_core=PER_CORE):
    nchunks = per_core // CHUNK
    assert nchunks * CHUNK == per_core

    nc = bacc.Bacc()
    featsT = nc.dram_tensor("featsT", (4, C, per_core), F32, kind="ExternalInput")
    attrsT2 = nc.dram_tensor("attrsT2", (2 * S, per_core), BF16, kind="ExternalInput")
    w0tab = nc.dram_tensor("w0tab", (2 * S, 5 * C), BF16, kind="ExternalInput")
    w1tab = nc.dram_tensor("w1tab", (2 * S, 4 * C), BF16, kind="ExternalInput")
    wl0 = nc.dram_tensor("wl0", (C, C), F32R, kind="ExternalInput")
    w1e = nc.dram_tensor("w1e", (C, 3 * 384), F32R, kind="ExternalInput")
    out = nc.dram_tensor("out", (per_core, 4 * C), F32, kind="ExternalOutput")

    with tile.TileContext(nc) as tc, ExitStack() as ctx:
        _body(ctx, tc, featsT, attrsT2, w0tab, w1tab, wl0, w1e, out, nchunks)
    nc.finalize()
    return nc


def _body(ctx, tc, featsT, attrsT2, w0tab, w1tab, wl0, w1e, out, nchunks):
    nc = tc.nc
    mult = mybir.AluOpType.mult
    add = mybir.AluOpType.add
    SQ = mybir.ActivationFunctionType.Square

    const = ctx.enter_context(tc.tile_pool(name="const", bufs=1))
    io = ctx.enter_context(tc.tile_pool(name="io", bufs=3))
    ew = ctx.enter_context(tc.tile_pool(name="ew", bufs=2))
    ewz = ctx.enter_context(tc.tile_pool(name="ewz", bufs=3))
    ps_w = ctx.enter_context(tc.tile_pool(name="ps_w", bufs=4, space="PSUM"))
    ps_y = ctx.enter_context(tc.tile_pool(name="ps_y", bufs=4, space="PSUM"))

    w0_sb = const.tile([2 * S, 5 * C], BF16)
    nc.sync.dma_start(out=w0_sb, in_=w0tab[:, :])
    w1_sb = const.tile([2 * S, 4 * C], BF16)
    nc.sync.dma_start(out=w1_sb, in_=w1tab[:, :])
    wl0_sb = const.tile([C, C], F32R)
    nc.sync.dma_start(out=wl0_sb, in_=wl0[:, :])
    w1e_sb = const.tile([C, 3 * 384], F32R)
    nc.sync.dma_start(out=w1e_sb, in_=w1e[:, :])

    fT2 = None
    for ci in range(nchunks):
        n0 = ci * CHUNK

        if ci % 2 == 0:
            span = min(2 * CHUNK, (nchunks - ci) * CHUNK)
            fT2 = io.tile([C, 4, 2 * CHUNK], F32, tag="fT2")
            nc.sync.dma_start(
                out=fT2[:, :, :span],
                in_=featsT[:, :, n0 : n0 + span].rearrange("k c n -> c k n"),
            )
            at22 = io.tile([2 * S, 2 * CHUNK], BF16, tag="at22")
            nc.gpsimd.dma_start(out=at22[:, :span], in_=attrsT2[:, n0 : n0 + span])
        half = (ci % 2) * CHUNK
        fT = fT2[:, :, half : half + CHUNK]
        at2 = at22[:, half : half + CHUNK]

        s = fT[:, 0, :]
        vx = fT[:, 1, :]
        vy = fT[:, 2, :]
        vz = fT[:, 3, :]

        # squares on ScalarE; |v|^2 sums on GPSIMD
        s2 = ew.tile([C, CHUNK], F32, tag="s2")
        nc.scalar.activation(out=s2, in_=s, func=SQ)
        vx2 = ew.tile([C, CHUNK], F32, tag="vx2")
        nc.scalar.activation(out=vx2, in_=vx, func=SQ)
        vy2 = ew.tile([C, CHUNK], F32, tag="vy2")
        nc.scalar.activation(out=vy2, in_=vy, func=SQ)
        vz2 = ew.tile([C, CHUNK], F32, tag="vz2")
        nc.scalar.activation(out=vz2, in_=vz, func=SQ)
        tv = ew.tile([C, CHUNK], F32, tag="tv")
        nc.gpsimd.tensor_tensor(out=tv, in0=vx2, in1=vy2, op=add)
        v2 = ew.tile([C, CHUNK], F32, tag="v2")
        nc.gpsimd.tensor_tensor(out=v2, in0=tv, in1=vz2, op=add)

        def path_mm(tab, p):
            w_ps = ps_w.tile([C, CHUNK], F32, tag="wps")
            nc.tensor.matmul(
                w_ps, lhsT=tab[:, bass.ts(p, C)], rhs=at2, start=True, stop=True
            )
            return w_ps

        # 1o channel
        w11 = path_mm(w1_sb, 1)
        R = ew.tile([C, CHUNK], F32, tag="R")
        nc.vector.tensor_tensor(out=R, in0=s, in1=w11, op=mult)
        w10 = path_mm(w1_sb, 0)
        R2 = ew.tile([C, CHUNK], F32, tag="R2")
        nc.vector.tensor_tensor(out=R2, in0=R, in1=w10, op=add)
        w12 = path_mm(w1_sb, 2)
        U = ew.tile([C, CHUNK], F32, tag="U")
        nc.vector.tensor_tensor(out=U, in0=s2, in1=w12, op=mult)
        R3 = ew.tile([C, CHUNK], F32, tag="R3")
        nc.vector.tensor_tensor(out=R3, in0=R2, in1=U, op=add)
        w13 = path_mm(w1_sb, 3)
        V = ew.tile([C, CHUNK], F32, tag="V")
        nc.vector.tensor_tensor(out=V, in0=v2, in1=w13, op=mult)
        c1 = ew.tile([C, CHUNK], F32, tag="c1")
        nc.vector.tensor_tensor(out=c1, in0=R3, in1=V, op=add)
        Z1x = ewz.tile([C, CHUNK], F32R, tag="Z1x")
        nc.vector.tensor_tensor(out=Z1x, in0=c1, in1=vx, op=mult)
        Z1y = ewz.tile([C, CHUNK], F32R, tag="Z1y")
        nc.vector.tensor_tensor(out=Z1y, in0=c1, in1=vy, op=mult)
        Z1z = ewz.tile([C, CHUNK], F32R, tag="Z1z")
        nc.vector.tensor_tensor(out=Z1z, in0=c1, in1=vz, op=mult)

        # 0e channel
        w03 = path_mm(w0_sb, 3)
        P = ew.tile([C, CHUNK], F32, tag="P")
        nc.vector.tensor_tensor(out=P, in0=s, in1=w03, op=mult)
        w01 = path_mm(w0_sb, 1)
        P2 = ew.tile([C, CHUNK], F32, tag="P2")
        nc.vector.tensor_tensor(out=P2, in0=P, in1=w01, op=add)
        w04 = path_mm(w0_sb, 4)
        Q = ew.tile([C, CHUNK], F32, tag="Q")
        nc.vector.tensor_tensor(out=Q, in0=s, in1=w04, op=mult)
        w02 = path_mm(w0_sb, 2)
        Q2 = ew.tile([C, CHUNK], F32, tag="Q2")
        nc.vector.tensor_tensor(out=Q2, in0=Q, in1=w02, op=add)
        w00 = path_mm(w0_sb, 0)
        Z_A = ewz.tile([C, CHUNK], F32R, tag="Z_A")
        nc.vector.tensor_tensor(out=Z_A, in0=s, in1=w00, op=mult)
        Z_B = ewz.tile([C, CHUNK], F32R, tag="Z_B")
        nc.vector.tensor_tensor(out=Z_B, in0=s2, in1=P2, op=mult)
        Z_C = ewz.tile([C, CHUNK], F32R, tag="Z_C")
        nc.gpsimd.tensor_tensor(out=Z_C, in0=v2, in1=Q2, op=mult)

        out_sb = io.tile([SUB, NSUB, 4 * C], F32, tag="out_sb")
        for t in range(NSUB):
            sl = bass.ts(t, SUB)
            y_ps = ps_y.tile([SUB, 4 * C], F32, tag="y")
            nc.tensor.matmul(y_ps[:, C:], lhsT=Z1x[:, sl], rhs=w1e_sb[:, 0:384],
                             start=True, stop=False)
            nc.tensor.matmul(y_ps[:, C:], lhsT=Z1y[:, sl], rhs=w1e_sb[:, 384:768],
                             start=False, stop=False)
            nc.tensor.matmul(y_ps[:, C:], lhsT=Z1z[:, sl], rhs=w1e_sb[:, 768:1152],
                             start=False, stop=True)
            nc.tensor.matmul(y_ps[:, 0:C], lhsT=Z_A[:, sl], rhs=wl0_sb,
                             start=True, stop=False)
            nc.tensor.matmul(y_ps[:, 0:C], lhsT=Z_B[:, sl], rhs=wl0_sb,
                             start=False, stop=False)
            nc.tensor.matmul(y_ps[:, 0:C], lhsT=Z_C[:, sl], rhs=wl0_sb,
                             start=False, stop=True)
            nc.scalar.copy(out=out_sb[:, t, :], in_=y_ps)

        nc.scalar.dma_start(
            out=out[n0 : n0 + CHUNK, :].rearrange("(t p) f -> p t f", p=SUB),
            in_=out_sb,
        )


_NC_CACHE = {}


def _get_nc(per_core):
    if per_core not in _NC_CACHE:
        _NC_CACHE[per_core] = build_bass(per_core)
    return _NC_CACHE[per_core]


def _prep_weights(w_paths0, w_paths1, W_lin0, W_lin1):
    def hilo(tab):  # [S, P*C] fp32 -> [2S, P*C] bf16 (hi, lo residual)
        hi = tab.astype(ml_dtypes.bfloat16)
        lo = (tab - hi.astype(np.float32)).astype(ml_dtypes.bfloat16)
        return np.concatenate([hi, lo], axis=0)

    w0tab = hilo(np.ascontiguousarray(w_paths0.reshape(S, 5 * C), np.float32))
    w1tab = hilo(np.ascontiguousarray(w_paths1.reshape(S, 4 * C), np.float32))
    wl0 = np.ascontiguousarray(W_lin0, dtype=np.float32)
    w1e = np.zeros((C, 3, 384), dtype=np.float32)
    for i in range(3):
        w1e[:, i, i::3] = W_lin1
    w1e = np.ascontiguousarray(w1e.reshape(C, 3 * 384))
    return w0tab, w1tab, wl0, w1e


def _prep_inputs(node_feats, node_attrs, per_core=PER_CORE):
    """Pad to 8*per_core nodes, channel-major feats planes, transposed attrs."""
    n = node_feats.shape[0]
    total = N_CORES * per_core
    featsT = np.zeros((N_CORES, 4, C, per_core), dtype=np.float32)
    attrsT2 = np.zeros((N_CORES, 2 * S, per_core), dtype=ml_dtypes.bfloat16)
    f = node_feats.reshape(n, C, 4)
    for k in range(N_CORES):
        r0 = k * per_core
        r1 = min(n, r0 + per_core)
        if r1 <= r0:
            continue
        m = r1 - r0
        featsT[k, :, :, :m] = f[r0:r1].transpose(2, 1, 0)
        aT = node_attrs[r0:r1].T.astype(ml_dtypes.bfloat16)
        attrsT2[k, :S, :m] = aT
        attrsT2[k, S:, :m] = aT
    return featsT, attrsT2


def kernel(node_feats, node_attrs, w_paths0, w_paths1, W_lin0, W_lin1):
    n = node_feats.shape[0]
    assert n == N_NODES, n
    featsT, attrsT2 = _prep_inputs(
        np.asarray(node_feats, np.float32), np.asarray(node_attrs, np.float32)
    )
    w0tab, w1tab, wl0, w1e = _prep_weights(w_paths0, w_paths1, W_lin0, W_lin1)

    nc = _get_nc(PER_CORE)
    in_maps = []
    for k in range(N_CORES):
        in_maps.append(
            {
                "featsT": featsT[k],
                "attrsT2": attrsT2[k],
                "w0tab": w0tab,
                "w1tab": w1tab,
                "wl0": wl0,
                "w1e": w1e,
            }
        )
    res = run_bass_kernel_spmd(nc, in_maps, core_ids=list(range(N_CORES)))
    outs = [res.results[k]["out"] for k in range(N_CORES)]
    full = np.concatenate(outs, axis=0)
    return np.ascontiguousarray(full[:N_NODES])



# revision 34
# speedup vs baseline: 2.4105x; 2.4105x over previous
"""
Trainium2 Bass kernel for nn_EquivariantProductBasisBlock.

Math (per node n, channel c):
    s   = feats[n,c,0];  v = feats[n,c,1:4]
    v2  = |v|^2 ;  s2 = s^2
    w0p[n,c] = w_paths0[spec(n),p,c]   (attrs are one-hot -> table lookup)
    w1p[n,c] = w_paths1[spec(n),p,c]
    out0 = s*w00 + s2*(w01 + s*w03) + v2*(w02 + s*w04)
    c1   = w10 + s*w11 + s2*w12 + v2*w13
    y0   = out0 @ W_lin0 ;  y1_i = (c1 * v_i) @ W_lin1
    out  = [y0 | interleave_i(y1_i)]

Key design (~2.6x faster than the matmul-path-weight baseline, 117 us):
  * Host sorts nodes by species and pads each species to SUPER=1280
    multiples, so every super-tile is species-uniform (80 supers total =
    10 per core, 2.4% padding).  The per-path species weights become
    per-partition scalars [128,1] (fp32, exact) fed to DVE tensor_scalar
    ops -- this deletes all 9 path-weight matmuls AND their 9 slow
    PSUM-reading vector ops per tile.  Species data rides in per-super
    wsel/w0sc input columns, so one SPMD program serves all cores.
  * All elementwise tiles are fp16 in SBUF: tensor_scalar ~4x DVE perf
    mode (474ns/1024), tensor_tensor 2x (685ns).  fp16 rounding ~1e-3
    rel, far inside the 2e-2 gate.
  * Host precomputes |v|^2 as a 5th input plane (cheaper in DMA than the
    5 on-chip ops it replaces; engines are the scarcer resource).
  * GPSIMD shares SBUF ports with DVE: concurrent Pool tensor ops slow
    DVE ~5x (measured), and GPSIMD cannot touch PSUM -- so Pool does
    nothing but const DMA issue.  Scalar-shaped work that must leave DVE
    goes to the Activation engine (Square; Identity(x*scale+bias) also
    works for fused scale+bias ops).
  * 0e path: w00/w01/w03 are folded into per-super pre-scaled W0
    matrices (host-side diag(w0p)@W_lin0), so y0 accumulates in PSUM as
    s@W0a + s2@W0b + s3@W0c + (v2*(s*w04+w02))@W0 -- the PE (idlest
    engine) absorbs three DVE products.
  * Final per-irrep linears: fp16 matmuls with N=128 (1 cycle/col; fp32r
    at N=128 pays a 4x penalty).  y1 written as contiguous blocks
    [y0|y1x|y1y|y1z]; the host interleaves columns for free.
  * Output fp16 (host upcasts): halves output DMA.  Act engine evicts
    PSUM->SBUF fp16 in [128,1024] copies; out-DMA flushed in thirds for
    pipeline drain.  PSUM pool at 3 bufs -- 4 bufs (16KB, the full PSUM)
    wedges the device.
  * Engine budget per 1280-node super: DMA ~2.9 MB (the bottleneck,
    ~82% active), DVE 10 ops ~7.4us total .. Act ~6.7, PE 70 matmuls.
"""

import sys

sys.path.insert(0, "/opt/trn_rl_repo")

from contextlib import ExitStack

import numpy as np

import concourse.bass as bass
import concourse.tile as tile
from concourse import bacc, mybir
from concourse.bass_utils import run_bass_kernel_spmd

N_CORES = 8
N_NODES = 100000
C = 128
S = 10
SUB = 128
NSUB = 10
SUPER = SUB * NSUB           # 1280 nodes per species-uniform tile
F32 = mybir.dt.float32
F16 = mybir.dt.float16


def build_bass(nsuper):
    per_core = nsuper * SUPER
    nc = bacc.Bacc()
    featsT = nc.dram_tensor("featsT", (5, C, per_core), F16, kind="ExternalInput")
    wsel = nc.dram_tensor("wsel", (C, nsuper * 9), F32, kind="ExternalInput")
    w0sc = nc.dram_tensor("w0sc", (C, nsuper * 3 * C), F16, kind="ExternalInput")
    wl0 = nc.dram_tensor("wl0", (C, C), F16, kind="ExternalInput")
    wl1 = nc.dram_tensor("wl1", (C, C), F16, kind="ExternalInput")
    out = nc.dram_tensor("out", (per_core, 4 * C), F16, kind="ExternalOutput")

    with tile.TileContext(nc) as tc, ExitStack() as ctx:
        _body(ctx, tc, featsT, wsel, w0sc, wl0, wl1, out, nsuper)
    nc.finalize()
    return nc


def _body(ctx, tc, featsT, wsel, w0sc, wl0, wl1, out, nsuper):
    nc = tc.nc
    mult = mybir.AluOpType.mult
    add = mybir.AluOpType.add
    SQ = mybir.ActivationFunctionType.Square
    IDN = mybir.ActivationFunctionType.Identity
    byp = mybir.AluOpType.bypass

    const = ctx.enter_context(tc.tile_pool(name="const", bufs=1))
    io = ctx.enter_context(tc.tile_pool(name="io", bufs=3))
    ob = ctx.enter_context(tc.tile_pool(name="ob", bufs=2))
    ew = ctx.enter_context(tc.tile_pool(name="ew", bufs=2))
    ewz = ctx.enter_context(tc.tile_pool(name="ewz", bufs=3))
    ps = ctx.enter_context(tc.tile_pool(name="ps", bufs=3, space="PSUM"))

    # const loads on the scalar queue (HWDGE, idle at t=0) so the first
    # feats DMA (sync queue) starts immediately
    wsel_sb = const.tile([C, nsuper * 9], F32)
    nc.scalar.dma_start(out=wsel_sb, in_=wsel[:, :])
    w0sc_sb = const.tile([C, nsuper * 3 * C], F16)
    nc.scalar.dma_start(out=w0sc_sb, in_=w0sc[:, :])
    wl0_sb = const.tile([C, C], F16)
    nc.scalar.dma_start(out=wl0_sb, in_=wl0[:, :])
    wl1_sb = const.tile([C, C], F16)
    nc.scalar.dma_start(out=wl1_sb, in_=wl1[:, :])

    # first load covers 1 super (short pipeline fill), then pairs
    fT2 = None
    for ci in range(nsuper):
        n0 = ci * SUPER

        if ci == 0 or ci % 2 == 1:
            span = SUPER if ci == 0 else min(2 * SUPER, (nsuper - ci) * SUPER)
            fT2 = io.tile([C, 5, 2 * SUPER], F16, tag="fT2", name="fT2")
            nc.sync.dma_start(
                out=fT2[:, :, :span],
                in_=featsT[:, :, n0 : n0 + span].rearrange("k c n -> c k n"),
            )
            half = 0
        else:
            half = SUPER
        s = fT2[:, 0, half : half + SUPER]
        vx = fT2[:, 1, half : half + SUPER]
        vy = fT2[:, 2, half : half + SUPER]
        vz = fT2[:, 3, half : half + SUPER]
        v2 = fT2[:, 4, half : half + SUPER]

        def wcol(j):
            return wsel_sb[:, ci * 9 + j : ci * 9 + j + 1]

        def t16(tag, pool=ew):
            return pool.tile([C, SUPER], F16, tag=tag, name=tag)

        # ---- Activation engine: square (own SBUF port; GPSIMD shares
        #      ports with DVE, so Pool must stay off tensor ops entirely)
        s2 = t16("s2", ewz)
        nc.scalar.activation(out=s2, in_=s, func=SQ)

        # ---- DVE ----
        # 1o: c1 = (s*w11 + w10) + s2*w12 + v2*w13
        g = t16("g")    # s*w11 + w10
        nc.vector.tensor_scalar(out=g, in0=s, scalar1=wcol(6), scalar2=wcol(5),
                                op0=mult, op1=add)
        u = t16("u")   # s2*w12
        nc.vector.tensor_scalar(out=u, in0=s2, scalar1=wcol(7), scalar2=None,
                                op0=mult)
        wv = t16("wv")  # v2*w13
        nc.vector.tensor_scalar(out=wv, in0=v2, scalar1=wcol(8), scalar2=None,
                                op0=mult)
        t1 = t16("t1")  # g + u
        nc.vector.tensor_tensor(out=t1, in0=g, in1=u, op=add)
        c1 = t16("c1")
        nc.vector.tensor_tensor(out=c1, in0=t1, in1=wv, op=add)
        # 0e: y0 = s@(w00*W0) + s2@(w01*W0) + s3@(w03*W0) + (v2*b)@W0
        #     (w00/w01/w03 folded into per-super pre-scaled W0 matrices)
        s3 = t16("s3", ewz)
        nc.vector.tensor_tensor(out=s3, in0=s2, in1=s, op=mult)
        b = t16("b")    # s*w04 + w02
        nc.vector.tensor_scalar(out=b, in0=s, scalar1=wcol(4), scalar2=wcol(2),
                                op0=mult, op1=add)
        ZC = t16("ZC", ewz)
        nc.vector.tensor_tensor(out=ZC, in0=v2, in1=b, op=mult)

        Z1x = t16("Z1x", ewz)
        nc.vector.tensor_tensor(out=Z1x, in0=c1, in1=vx, op=mult)
        Z1y = t16("Z1y", ewz)
        nc.vector.tensor_tensor(out=Z1y, in0=c1, in1=vy, op=mult)
        Z1z = t16("Z1z", ewz)
        nc.vector.tensor_tensor(out=Z1z, in0=c1, in1=vz, op=mult)

        # ---- final per-irrep linears + eviction ----
        w0a = w0sc_sb[:, (ci * 3 + 0) * C : (ci * 3 + 1) * C]
        w0b = w0sc_sb[:, (ci * 3 + 1) * C : (ci * 3 + 2) * C]
        w0d = w0sc_sb[:, (ci * 3 + 2) * C : (ci * 3 + 3) * C]
        out_sb = ob.tile([SUB, NSUB, 4 * C], F16, tag="out_sb")
        for h in range(NSUB // 2):
            y4 = ps.tile([SUB, 2, 4 * C], F32, tag="y")
            for q in range(2):
                t = 2 * h + q
                sl = bass.ts(t, SUB)
                nc.tensor.matmul(y4[:, q, 0:C], lhsT=s[:, sl], rhs=w0a,
                                 start=True, stop=False)
                nc.tensor.matmul(y4[:, q, 0:C], lhsT=s2[:, sl], rhs=w0b,
                                 start=False, stop=False)
                nc.tensor.matmul(y4[:, q, 0:C], lhsT=s3[:, sl], rhs=w0d,
                                 start=False, stop=False)
                nc.tensor.matmul(y4[:, q, 0:C], lhsT=ZC[:, sl], rhs=wl0_sb,
                                 start=False, stop=True)
                nc.tensor.matmul(y4[:, q, C:2 * C], lhsT=Z1x[:, sl], rhs=wl1_sb,
                                 start=True, stop=True)
                nc.tensor.matmul(y4[:, q, 2 * C:3 * C], lhsT=Z1y[:, sl], rhs=wl1_sb,
                                 start=True, stop=True)
                nc.tensor.matmul(y4[:, q, 3 * C:4 * C], lhsT=Z1z[:, sl], rhs=wl1_sb,
                                 start=True, stop=True)
            nc.scalar.copy(out=out_sb[:, 2 * h : 2 * h + 2, :], in_=y4)
            # flush evicted subtile groups to DRAM as they complete
            if h == 1 or h == 3 or h == NSUB // 2 - 1:
                lo = 0 if h == 1 else (4 if h == 3 else 8)
                hi = 2 * h + 2
                nc.gpsimd.dma_start(
                    out=out[n0 + lo * SUB : n0 + hi * SUB, :].rearrange(
                        "(t p) f -> p t f", p=SUB
                    ),
                    in_=out_sb[:, lo:hi, :],
                )


_NC_CACHE = {}


def _get_nc(nsuper):
    if nsuper not in _NC_CACHE:
        _NC_CACHE[nsuper] = build_bass(nsuper)
    return _NC_CACHE[nsuper]


def kernel(node_feats, node_attrs, w_paths0, w_paths1, W_lin0, W_lin1):
    n = node_feats.shape[0]
    assert n == N_NODES, n

    species = np.argmax(np.asarray(node_attrs), axis=1).astype(np.int64)
    counts = np.bincount(species, minlength=S)
    sup_sp = -(-counts // SUPER)                       # supers per species
    total_sup = int(sup_sp.sum())
    T = -(-total_sup // N_CORES) * N_CORES             # pad to multiple of 8
    nsuper = T // N_CORES
    padded_n = T * SUPER
    per_core = nsuper * SUPER

    # destination slot (species-sorted, super-padded) for each node
    off = np.zeros(S, np.int64)
    off[1:] = np.cumsum(sup_sp * SUPER)[:-1]
    order = np.argsort(species, kind="stable")
    dst = np.empty(n, np.int64)
    pos = 0
    for sp in range(S):
        n_s = int(counts[sp])
        dst[order[pos : pos + n_s]] = off[sp] + np.arange(n_s)
        pos += n_s

    # species of each super tile (padding supers read species 0 weights)
    sup_species = np.zeros(T, np.int64)
    sup_species[:total_sup] = np.repeat(np.arange(S), sup_sp)

    # input planes [5, C, padded_n] fp16: s, vx, vy, vz, |v|^2
    f = np.asarray(node_feats, np.float32)
    planes = np.zeros((5, C, padded_n), np.float16)
    planes[0][:, dst] = f[:, :, 0].T
    planes[1][:, dst] = f[:, :, 1].T
    planes[2][:, dst] = f[:, :, 2].T
    planes[3][:, dst] = f[:, :, 3].T
    v2 = f[:, :, 1] ** 2 + f[:, :, 2] ** 2 + f[:, :, 3] ** 2
    planes[4][:, dst] = v2.T

    # per-super path-weight scalars [C, T*9] fp32
    w0 = np.asarray(w_paths0, np.float32)              # [S, 5, C]
    w1 = np.asarray(w_paths1, np.float32)              # [S, 4, C]
    wtab = np.concatenate([w0, w1], axis=1)            # [S, 9, C]
    wsel = wtab[sup_species].transpose(2, 0, 1).reshape(C, T * 9)

    # per-super pre-scaled W0 matrices for the s/s2/s3 accumulation terms
    W0f = np.asarray(W_lin0, np.float32)               # [C, C]
    w0m = w0[:, [0, 1, 3], :, None] * W0f[None, None]  # [S, 3, C, C]
    w0sc = (
        w0m[sup_species]                               # [T, 3, C, C]
        .transpose(2, 0, 1, 3).reshape(C, T * 3 * C).astype(np.float16)
    )

    wl0 = np.asarray(W_lin0, np.float16)
    wl1 = np.asarray(W_lin1, np.float16)

    nc = _get_nc(nsuper)
    in_maps = []
    for k in range(N_CORES):
        c0 = k * per_core
        in_maps.append(
            {
                "featsT": np.ascontiguousarray(planes[:, :, c0 : c0 + per_core]),
                "wsel": np.ascontiguousarray(
                    wsel[:, k * nsuper * 9 : (k + 1) * nsuper * 9]
                ),
                "w0sc": np.ascontiguousarray(
                    w0sc[:, k * nsuper * 3 * C : (k + 1) * nsuper * 3 * C]
                ),
                "wl0": wl0,
                "wl1": wl1,
            }
        )
    res = run_bass_kernel_spmd(nc, in_maps, core_ids=list(range(N_CORES)))
    outs = [res.results[k]["out"] for k in range(N_CORES)]
    full = np.concatenate(outs, axis=0)[dst].astype(np.float32)  # [n, 512]

    y0 = full[:, :C]
    y1 = full[:, C:].reshape(n, 3, C).transpose(0, 2, 1).reshape(n, 3 * C)
    return np.ascontiguousarray(np.concatenate([y0, y1], axis=1))


# revision 35
# speedup vs baseline: 2.7405x; 1.1369x over previous
"""
Trainium2 Bass kernel for nn_EquivariantProductBasisBlock.

Math (per node n, channel c):
    s   = feats[n,c,0];  v = feats[n,c,1:4]
    v2  = |v|^2 ;  s2 = s^2
    w0p[n,c] = w_paths0[spec(n),p,c]   (attrs are one-hot -> table lookup)
    w1p[n,c] = w_paths1[spec(n),p,c]
    out0 = s*w00 + s2*(w01 + s*w03) + v2*(w02 + s*w04)
    c1   = w10 + s*w11 + s2*w12 + v2*w13
    y0   = out0 @ W_lin0 ;  y1_i = (c1 * v_i) @ W_lin1
    out  = [y0 | interleave_i(y1_i)]

Key design (~2.6x faster than the matmul-path-weight baseline, 117 us):
  * Host sorts nodes by species and pads each species to SUPER=1280
    multiples, so every super-tile is species-uniform (80 supers total =
    10 per core, 2.4% padding).  The per-path species weights become
    per-partition scalars [128,1] (fp32, exact) fed to DVE tensor_scalar
    ops -- this deletes all 9 path-weight matmuls AND their 9 slow
    PSUM-reading vector ops per tile.  Species data rides in per-super
    wsel/w0sc input columns, so one SPMD program serves all cores.
  * All elementwise tiles are fp16 in SBUF: tensor_scalar ~4x DVE perf
    mode (474ns/1024), tensor_tensor 2x (685ns).  fp16 rounding ~1e-3
    rel, far inside the 2e-2 gate.
  * Host precomputes |v|^2 as a 5th input plane (cheaper in DMA than the
    5 on-chip ops it replaces; engines are the scarcer resource).
  * GPSIMD shares SBUF ports with DVE: concurrent Pool tensor ops slow
    DVE ~5x (measured), and GPSIMD cannot touch PSUM -- so Pool does
    nothing but const DMA issue.  Scalar-shaped work that must leave DVE
    goes to the Activation engine (Square; Identity(x*scale+bias) also
    works for fused scale+bias ops).
  * 0e path: w00/w01/w03 are folded into per-super pre-scaled W0
    matrices (host-side diag(w0p)@W_lin0), so y0 accumulates in PSUM as
    s@W0a + s2@W0b + s3@W0c + (v2*(s*w04+w02))@W0 -- the PE (idlest
    engine) absorbs three DVE products.
  * Final per-irrep linears: fp16 matmuls with N=128 (1 cycle/col; fp32r
    at N=128 pays a 4x penalty).  y1 written as contiguous blocks
    [y0|y1x|y1y|y1z]; the host interleaves columns for free.
  * Output fp16 (host upcasts): halves output DMA.  Act engine evicts
    PSUM->SBUF fp16 in [128,1024] copies; out-DMA flushed in thirds for
    pipeline drain.  PSUM pool at 3 bufs -- 4 bufs (16KB, the full PSUM)
    wedges the device.
  * Engine budget per 1280-node super: DMA ~2.9 MB (the bottleneck,
    ~82% active), DVE 10 ops ~7.4us total .. Act ~6.7, PE 70 matmuls.
"""

import sys

sys.path.insert(0, "/opt/trn_rl_repo")

from contextlib import ExitStack

import numpy as np

import concourse.bass as bass
import concourse.tile as tile
from concourse import bacc, mybir
from concourse.bass_utils import run_bass_kernel_spmd

N_CORES = 8
N_NODES = 100000
C = 128
S = 10
SUB = 128
NSUB = 10
SUPER = SUB * NSUB           # 1280 nodes per species-uniform tile
F32 = mybir.dt.float32
F16 = mybir.dt.float16


def build_bass(nsuper):
    per_core = nsuper * SUPER
    nc = bacc.Bacc()
    featsT = nc.dram_tensor("featsT", (5, C, per_core), F16, kind="ExternalInput")
    wsel = nc.dram_tensor("wsel", (C, nsuper * 9), F32, kind="ExternalInput")
    w0sc = nc.dram_tensor("w0sc", (C, nsuper * 3 * C), F16, kind="ExternalInput")
    wl0 = nc.dram_tensor("wl0", (C, C), F16, kind="ExternalInput")
    wl1 = nc.dram_tensor("wl1", (C, C), F16, kind="ExternalInput")
    out = nc.dram_tensor("out", (per_core, 4 * C), F16, kind="ExternalOutput")

    with tile.TileContext(nc) as tc, ExitStack() as ctx:
        _body(ctx, tc, featsT, wsel, w0sc, wl0, wl1, out, nsuper)
    nc.finalize()
    return nc


def _body(ctx, tc, featsT, wsel, w0sc, wl0, wl1, out, nsuper):
    nc = tc.nc
    mult = mybir.AluOpType.mult
    add = mybir.AluOpType.add
    SQ = mybir.ActivationFunctionType.Square
    IDN = mybir.ActivationFunctionType.Identity
    byp = mybir.AluOpType.bypass

    const = ctx.enter_context(tc.tile_pool(name="const", bufs=1))
    io = ctx.enter_context(tc.tile_pool(name="io", bufs=3))
    ob = ctx.enter_context(tc.tile_pool(name="ob", bufs=2))
    ew = ctx.enter_context(tc.tile_pool(name="ew", bufs=2))
    ewz = ctx.enter_context(tc.tile_pool(name="ewz", bufs=3))
    ps = ctx.enter_context(tc.tile_pool(name="ps", bufs=3, space="PSUM"))

    # const loads on the scalar queue (HWDGE, idle at t=0) so the first
    # feats DMA (sync queue) starts immediately
    wsel_sb = const.tile([C, nsuper * 9], F32)
    nc.scalar.dma_start(out=wsel_sb, in_=wsel[:, :])
    w0sc_sb = const.tile([C, nsuper * 3 * C], F16)
    nc.scalar.dma_start(out=w0sc_sb, in_=w0sc[:, :])
    wl0_sb = const.tile([C, C], F16)
    nc.scalar.dma_start(out=wl0_sb, in_=wl0[:, :])
    wl1_sb = const.tile([C, C], F16)
    nc.scalar.dma_start(out=wl1_sb, in_=wl1[:, :])

    # first load covers 1 super (short pipeline fill), then pairs
    fT2 = None
    for ci in range(nsuper):
        n0 = ci * SUPER

        if ci == 0 or ci % 2 == 1:
            span = SUPER if ci == 0 else min(2 * SUPER, (nsuper - ci) * SUPER)
            fT2 = io.tile([C, 5, 2 * SUPER], F16, tag="fT2", name="fT2")
            nc.sync.dma_start(
                out=fT2[:, :, :span],
                in_=featsT[:, :, n0 : n0 + span].rearrange("k c n -> c k n"),
            )
            half = 0
        else:
            half = SUPER
        s = fT2[:, 0, half : half + SUPER]
        vx = fT2[:, 1, half : half + SUPER]
        vy = fT2[:, 2, half : half + SUPER]
        vz = fT2[:, 3, half : half + SUPER]
        v2 = fT2[:, 4, half : half + SUPER]

        def wcol(j):
            return wsel_sb[:, ci * 9 + j : ci * 9 + j + 1]

        def t16(tag, pool=ew):
            return pool.tile([C, SUPER], F16, tag=tag, name=tag)

        # ---- Activation engine: square (own SBUF port; GPSIMD shares
        #      ports with DVE, so Pool must stay off tensor ops entirely)
        s2 = t16("s2", ewz)
        nc.scalar.activation(out=s2, in_=s, func=SQ)

        # ---- DVE ----
        # 1o: c1 = (s*w11 + w10) + s2*w12 + v2*w13
        g = t16("g")    # s*w11 + w10
        nc.vector.tensor_scalar(out=g, in0=s, scalar1=wcol(6), scalar2=wcol(5),
                                op0=mult, op1=add)
        u = t16("u")   # s2*w12
        nc.vector.tensor_scalar(out=u, in0=s2, scalar1=wcol(7), scalar2=None,
                                op0=mult)
        wv = t16("wv")  # v2*w13
        nc.vector.tensor_scalar(out=wv, in0=v2, scalar1=wcol(8), scalar2=None,
                                op0=mult)
        t1 = t16("t1")  # g + u
        nc.vector.tensor_tensor(out=t1, in0=g, in1=u, op=add)
        c1 = t16("c1")
        nc.vector.tensor_tensor(out=c1, in0=t1, in1=wv, op=add)
        # 0e: y0 = s@(w00*W0) + s2@(w01*W0) + s3@(w03*W0) + (v2*b)@W0
        #     (w00/w01/w03 folded into per-super pre-scaled W0 matrices)
        s3 = t16("s3", ewz)
        nc.vector.tensor_tensor(out=s3, in0=s2, in1=s, op=mult)
        b = t16("b")    # s*w04 + w02
        nc.vector.tensor_scalar(out=b, in0=s, scalar1=wcol(4), scalar2=wcol(2),
                                op0=mult, op1=add)
        ZC = t16("ZC", ewz)
        nc.vector.tensor_tensor(out=ZC, in0=v2, in1=b, op=mult)

        Z1x = t16("Z1x", ewz)
        nc.vector.tensor_tensor(out=Z1x, in0=c1, in1=vx, op=mult)
        Z1y = t16("Z1y", ewz)
        nc.vector.tensor_tensor(out=Z1y, in0=c1, in1=vy, op=mult)
        Z1z = t16("Z1z", ewz)
        nc.vector.tensor_tensor(out=Z1z, in0=c1, in1=vz, op=mult)

        # ---- final per-irrep linears + eviction ----
        w0a = w0sc_sb[:, (ci * 3 + 0) * C : (ci * 3 + 1) * C]
        w0b = w0sc_sb[:, (ci * 3 + 1) * C : (ci * 3 + 2) * C]
        w0d = w0sc_sb[:, (ci * 3 + 2) * C : (ci * 3 + 3) * C]
        out_sb = ob.tile([SUB, NSUB, 4 * C], F16, tag="out_sb")
        for h in range(NSUB // 2):
            y4 = ps.tile([SUB, 2, 4 * C], F32, tag="y")
            for q in range(2):
                t = 2 * h + q
                sl = bass.ts(t, SUB)
                nc.tensor.matmul(y4[:, q, 0:C], lhsT=s[:, sl], rhs=w0a,
                                 start=True, stop=False)
                nc.tensor.matmul(y4[:, q, 0:C], lhsT=s2[:, sl], rhs=w0b,
                                 start=False, stop=False)
                nc.tensor.matmul(y4[:, q, 0:C], lhsT=s3[:, sl], rhs=w0d,
                                 start=False, stop=False)
                nc.tensor.matmul(y4[:, q, 0:C], lhsT=ZC[:, sl], rhs=wl0_sb,
                                 start=False, stop=True)
                nc.tensor.matmul(y4[:, q, C:2 * C], lhsT=Z1x[:, sl], rhs=wl1_sb,
                                 start=True, stop=True)
                nc.tensor.matmul(y4[:, q, 2 * C:3 * C], lhsT=Z1y[:, sl], rhs=wl1_sb,
                                 start=True, stop=True)
                nc.tensor.matmul(y4[:, q, 3 * C:4 * C], lhsT=Z1z[:, sl], rhs=wl1_sb,
                                 start=True, stop=True)
            nc.scalar.copy(out=out_sb[:, 2 * h : 2 * h + 2, :], in_=y4)
            # flush evicted subtile groups to DRAM as they complete
            if h == 1 or h == 3 or h == NSUB // 2 - 1:
                lo = 0 if h == 1 else (4 if h == 3 else 8)
                hi = 2 * h + 2
                nc.sync.dma_start(
                    out=out[n0 + lo * SUB : n0 + hi * SUB, :].rearrange(
                        "(t p) f -> p t f", p=SUB
                    ),
                    in_=out_sb[:, lo:hi, :],
                )


_NC_CACHE = {}


def _get_nc(nsuper):
    if nsuper not in _NC_CACHE:
        _NC_CACHE[nsuper] = build_bass(nsuper)
    return _NC_CACHE[nsuper]


def kernel(node_feats, node_attrs, w_paths0, w_paths1, W_lin0, W_lin1):
    n = node_feats.shape[0]
    assert n == N_NODES, n

    species = np.argmax(np.asarray(node_attrs), axis=1).astype(np.int64)
    counts = np.bincount(species, minlength=S)
    sup_sp = -(-counts // SUPER)                       # supers per species
    total_sup = int(sup_sp.sum())
    T = -(-total_sup // N_CORES) * N_CORES             # pad to multiple of 8
    nsuper = T // N_CORES
    padded_n = T * SUPER
    per_core = nsuper * SUPER

    # destination slot (species-sorted, super-padded) for each node
    off = np.zeros(S, np.int64)
    off[1:] = np.cumsum(sup_sp * SUPER)[:-1]
    order = np.argsort(species, kind="stable")
    dst = np.empty(n, np.int64)
    pos = 0
    for sp in range(S):
        n_s = int(counts[sp])
        dst[order[pos : pos + n_s]] = off[sp] + np.arange(n_s)
        pos += n_s

    # species of each super tile (padding supers read species 0 weights)
    sup_species = np.zeros(T, np.int64)
    sup_species[:total_sup] = np.repeat(np.arange(S), sup_sp)

    # input planes [5, C, padded_n] fp16: s, vx, vy, vz, |v|^2
    f = np.asarray(node_feats, np.float32)
    planes = np.zeros((5, C, padded_n), np.float16)
    planes[0][:, dst] = f[:, :, 0].T
    planes[1][:, dst] = f[:, :, 1].T
    planes[2][:, dst] = f[:, :, 2].T
    planes[3][:, dst] = f[:, :, 3].T
    v2 = f[:, :, 1] ** 2 + f[:, :, 2] ** 2 + f[:, :, 3] ** 2
    planes[4][:, dst] = v2.T

    # per-super path-weight scalars [C, T*9] fp32
    w0 = np.asarray(w_paths0, np.float32)              # [S, 5, C]
    w1 = np.asarray(w_paths1, np.float32)              # [S, 4, C]
    wtab = np.concatenate([w0, w1], axis=1)            # [S, 9, C]
    wsel = wtab[sup_species].transpose(2, 0, 1).reshape(C, T * 9)

    # per-super pre-scaled W0 matrices for the s/s2/s3 accumulation terms
    W0f = np.asarray(W_lin0, np.float32)               # [C, C]
    w0m = w0[:, [0, 1, 3], :, None] * W0f[None, None]  # [S, 3, C, C]
    w0sc = (
        w0m[sup_species]                               # [T, 3, C, C]
        .transpose(2, 0, 1, 3).reshape(C, T * 3 * C).astype(np.float16)
    )

    wl0 = np.asarray(W_lin0, np.float16)
    wl1 = np.asarray(W_lin1, np.float16)

    nc = _get_nc(nsuper)
    in_maps = []
    for k in range(N_CORES):
        c0 = k * per_core
        in_maps.append(
            {
                "featsT": np.ascontiguousarray(planes[:, :, c0 : c0 + per_core]),
                "wsel": np.ascontiguousarray(
                    wsel[:, k * nsuper * 9 : (k + 1) * nsuper * 9]
                ),
                "w0sc": np.ascontiguousarray(
                    w0sc[:, k * nsuper * 3 * C : (k + 1) * nsuper * 3 * C]
                ),
                "wl0": wl0,
                "wl1": wl1,
            }
        )
    res = run_bass_kernel_spmd(nc, in_maps, core_ids=list(range(N_CORES)))
    outs = [res.results[k]["out"] for k in range(N_CORES)]
    full = np.concatenate(outs, axis=0)[dst].astype(np.float32)  # [n, 512]

    y0 = full[:, :C]
    y1 = full[:, C:].reshape(n, 3, C).transpose(0, 2, 1).reshape(n, 3 * C)
    return np.ascontiguousarray(np.concatenate([y0, y1], axis=1))
